# revision 14
# baseline (speedup 1.0000x reference)
"""3D Haar DWT (single level) on 8 Trainium2 NeuronCores.

Input x: (2, 4, 128, 256, 256) f32. Output: 8 subbands (LLL..HHH), each
(2, 4, 64, 128, 128) f32.

Sharding: pure data parallel - B*C = 8 independent (128, 256, 256) volumes,
one per core. No cross-core communication.

Design notes:
- All I/O in fp16: the host pre-scales x by (1/sqrt2)^3 and casts to fp16,
  so the device only does pairwise adds/subs (3 separable Haar stages) and
  both the host<->device streams and HBM traffic are halved vs f32.
- Partition layout p = (c2, dp): c2 = h-half (2), dp = d-pair index (64).
  All three Haar stages are lane-local DVE tensor_add/tensor_sub:
    stage D: pairs along d sit in the free dim (e axis of the input tile)
    stage H: pairs along h (stride-2 rows inside the 16-row chunk)
    stage W: pairs along w (stride-2 in the innermost dim)
- 8 h-chunks of 16 rows; per chunk one 2 MiB input DMA (128 descriptors x
  8 KiB, SP HWDGE ring) and one 2 MiB output DMA (128 x 16 KiB fully
  contiguous, ACT HWDGE ring). Host reassembles the 8 subbands.
"""

import sys

sys.path.insert(0, "/opt/trn_rl_repo")

import json

import numpy as np

import concourse.bass as bass
import concourse.mybir as mybir
import concourse.tile as tile
from concourse import bass_utils

_C3 = np.float32(1.0 / (2.0 * np.sqrt(2.0)))  # (1/sqrt2)^3, folded into the input

# ---------------------------------------------------------------------------
# BIR post-pass: this walrus build has tight per-instruction sync-wait
# encoding limits (Drain/TPB_CTRL: 0 waits; everything else observed to
# reject 2+ waits: Matmult/S3_LW, DMACopy, TensorTensor). Keep at most one
# wait per instruction and hoist the excess onto EventSemaphore instructions
# inserted right before it on the same engine - program order makes that
# equivalent.
# ---------------------------------------------------------------------------
_MAX_WAITS = {"Drain": 0}
_DEFAULT_MAX_WAITS = 1


def _fix_sync_limits(bir_bytes: bytes) -> bytes:
    m = json.loads(bir_bytes)

    def fix_block(blk):
        insts = blk.get("instructions", [])
        new = []
        for i in insts:
            limit = _MAX_WAITS.get(i.get("opcode"), _DEFAULT_MAX_WAITS)
            si = i.get("sync_info") or {}
            waits = si.get("on_wait") or []
            if len(waits) > limit:
                n_hoist = len(waits) - limit
                for wi, w in enumerate(waits[:n_hoist]):
                    ev = {
                        "name": i["name"] + f"-hoistwait{wi}",
                        "opcode": "EventSemaphore",
                        "engine": i["engine"],
                        "ins": [],
                        "outs": [],
                        "sync_info": {"on_wait": [w], "on_update": []},
                    }
                    if "debug" in i:
                        ev["debug"] = i["debug"]
                    new.append(ev)
                si = dict(si)
                si["on_wait"] = waits[n_hoist:]
                i = dict(i)
                i["sync_info"] = si
            new.append(i)
        blk["instructions"] = new
        for sub in blk.get("blocks", []):
            fix_block(sub)

    for f in m["functions"]:
        for blk in f["blocks"]:
            fix_block(blk)
    return json.dumps(m).encode()


_patched = False


def _install_patch():
    global _patched
    if _patched:
        return
    orig = bass.Bass.to_json_bytes

    def patched(self, *a, **k):
        return _fix_sync_limits(orig(self, *a, **k))

    bass.Bass.to_json_bytes = patched
    _patched = True


_PROGRAM = None


def _build_program(reps: int = 1, mode: str = "full", wsplit: int = 0) -> bass.Bass:
    """reps>1 wraps the whole pipeline in a dynamic loop (benchmarking only).

    mode="dmaonly" drops the compute (out-DMA reads the input tile) to
    measure the pure DMA pipeline. wsplit=q moves the last q of the 4
    q-rows of both W-stage ops onto the GpSimd engine (0 = all on DVE).
    """
    global _PROGRAM
    if reps == 1 and mode == "full" and wsplit == 0 and _PROGRAM is not None:
        return _PROGRAM
    _install_patch()

    F16 = mybir.dt.float16
    nc = bass.Bass()
    # x: (d=128, h=256, w=256) fp16, pre-scaled by (1/sqrt2)^3 on the host.
    x = nc.dram_tensor("x", [128, 256, 256], F16, kind="ExternalInput")
    # y dims: (c2, dp, hc, kD, kH, hp, kW, wp) - per partition (c2, dp) and
    # chunk hc the whole (kD kH hp kW wp) block is one contiguous 32 KiB run.
    y = nc.dram_tensor("y", [2, 64, 4, 2, 2, 16, 2, 128], F16, kind="ExternalOutput")

    # d = 2*dp + e ; h = 128*c2 + 32*hc + h32 ; partition = 64*c2 + dp
    xv = x.rearrange("(dp e) (c2 hc h) w -> hc c2 dp e h w", e=2, c2=2, hc=4, h=32)
    yv = y.rearrange("c2 dp hc kD kH hp kW wp -> hc (c2 dp) (kD kH hp kW wp)")

    with tile.TileContext(nc) as tc:
        with (
            tc.tile_pool(name="xin", bufs=2) as xpool,
            tc.tile_pool(name="dst", bufs=1) as dpool,
            tc.tile_pool(name="hst", bufs=1) as hpool,
            tc.tile_pool(name="outp", bufs=2) as opool,
        ):

            def run_chunks():
                for hc in range(4):
                    X = xpool.tile([128, 16384], F16, tag="X")
                    for c2 in range(2):
                        nc.sync.dma_start(
                            out=X[64 * c2 : 64 * c2 + 64, :].rearrange(
                                "dp (e h w) -> dp e h w", e=2, h=32
                            ),
                            in_=xv[hc, c2],
                        )

                    if mode == "dmaonly":
                        nc.scalar.dma_start(out=yv[hc], in_=X[:])
                        continue

                    # stage D: pairs along d (the e axis of the input tile)
                    # D cols: (kD, hp, eh, w)
                    Xe = X[:].rearrange("p (e f) -> p e f", e=2)
                    D = dpool.tile([128, 16384], F16, tag="D")
                    Dv = D[:].rearrange("p (kD f) -> p kD f", kD=2)
                    nc.vector.tensor_add(out=Dv[:, 0], in0=Xe[:, 0], in1=Xe[:, 1])
                    nc.vector.tensor_sub(out=Dv[:, 1], in0=Xe[:, 0], in1=Xe[:, 1])

                    # stage H: pairs along h -> HT cols (kD, kH, hp, w)
                    HT = hpool.tile([128, 16384], F16, tag="HT")
                    HTv = HT[:].rearrange("p (kD kH hp w) -> p kD kH hp w", kD=2, kH=2, hp=16)
                    sv = D[:].rearrange("p (kD hp eh w) -> p kD hp eh w", kD=2, hp=16, eh=2)
                    nc.vector.tensor_add(
                        out=HTv[:, :, 0], in0=sv[:, :, :, 0], in1=sv[:, :, :, 1]
                    )
                    nc.vector.tensor_sub(
                        out=HTv[:, :, 1], in0=sv[:, :, :, 0], in1=sv[:, :, :, 1]
                    )

                    # stage W: pairs along w -> O cols (kD, kH, hp, kW, wp)
                    O = opool.tile([128, 16384], F16, tag="O")
                    Ov = O[:].rearrange("p (q hp kW wp) -> p q hp kW wp", q=4, hp=16, kW=2)
                    Hv = HT[:].rearrange("p (q hp wp ew) -> p q hp wp ew", q=4, hp=16, ew=2)
                    qd = 4 - wsplit  # first qd q-rows on DVE, rest on gpsimd
                    if qd > 0:
                        nc.vector.tensor_add(
                            out=Ov[:, :qd, :, 0],
                            in0=Hv[:, :qd, :, :, 0],
                            in1=Hv[:, :qd, :, :, 1],
                        )
                        nc.vector.tensor_sub(
                            out=Ov[:, :qd, :, 1],
                            in0=Hv[:, :qd, :, :, 0],
                            in1=Hv[:, :qd, :, :, 1],
                        )
                    if wsplit > 0:
                        nc.gpsimd.tensor_add(
                            out=Ov[:, qd:, :, 0],
                            in0=Hv[:, qd:, :, :, 0],
                            in1=Hv[:, qd:, :, :, 1],
                        )
                        nc.gpsimd.tensor_sub(
                            out=Ov[:, qd:, :, 1],
                            in0=Hv[:, qd:, :, :, 0],
                            in1=Hv[:, qd:, :, :, 1],
                        )

                    nc.scalar.dma_start(out=yv[hc], in_=O[:])

            if reps == 1:
                run_chunks()
            else:
                with tc.For_i(0, reps, 1):
                    run_chunks()

    if reps == 1 and mode == "full" and wsplit == 0:
        _PROGRAM = nc
    return nc


def kernel(x: np.ndarray):
    x = np.asarray(x)
    assert x.shape == (2, 4, 128, 256, 256)
    nc = _build_program()

    xs = (np.asarray(x, np.float32).reshape(8, 128, 256, 256) * _C3).astype(
        np.float16
    )
    in_maps = [{"x": xs[i]} for i in range(8)]
    res = bass_utils.run_bass_kernel_spmd(
        nc, in_maps, core_ids=list(range(8)), trace=False
    )

    bands = np.empty((8, 2, 4, 64, 128, 128), np.float32)
    for i in range(8):
        yc = res.results[i]["y"]  # (c2, dp, hc, kD, kH, hp, kW, wp) fp16
        # -> (kD, kH, kW, dp, c2, hc, hp, wp) -> (8, 64, 128, 128)
        bands[:, i // 4, i % 4] = (
            yc.transpose(3, 4, 6, 1, 0, 2, 5, 7)
            .reshape(8, 64, 128, 128)
            .astype(np.float32)
        )
    return tuple(bands[s] for s in range(8))
